# revision 17
# baseline (speedup 1.0000x reference)
"""Banded local-linear layer (nn_LocalLinearLayer) on 8 trn2 NeuronCores.

out[b, o, c] = sum_p W[o, p] * xpad[b, c, p] + bias[o],  band p in [o, o+25)
xpad = edge-replicate pad of x along L (first/last 12 rows duplicated).

Strategy (v11):
  - Tensor-parallel over L: 40 global output tiles of 104 rows (K=128 window);
    core s owns tiles [5s, 5s+5) and only its slice of the banded weight.
  - The per-tile weight block [128, 104] is PACKED at the head of the x tile
    ([104 w | 2048 x] = 4304 B lines), so each tile arrives in one large-line
    DMA. Tiles 0 and 4 are split BY PARTITION across both HW queues so the
    first matmul starts early and the input stream ends early.
  - Warmup matmuls on rotating PSUM banks keep the PE clock ramping before
    the first x tile lands.
  - Per tile: 4 matmuls (N=512, one PSUM bank each; bufs=8 via two pools of
    [104,1024]) drained per 1024 cols by a pure copy (vector/scalar
    alternating) into paired fp16 out tiles (8KB DRAM lines for the slow
    write path). Bias is added on the HOST during gather.
  - Out rings: tiles01 -> gpsimd, tiles23 -> sync, tile4 partition-split on
    sync+scalar.
  - fp16 operands and output, fp32 PSUM.
"""

import sys

for _p in ("/opt/trn_rl_repo",):
    if _p not in sys.path:
        sys.path.insert(0, _p)

import numpy as np

import concourse.bass as bass
import concourse.tile as tile
from concourse import bacc, mybir
from concourse.bass_utils import run_bass_kernel_spmd

L = 4096
WIN = 25
PAD = (WIN - 1) // 2  # 12
PADDED = L + 2 * PAD  # 4120
B = 32
C = 64
NCORES = 8
P = 128
M = P - (WIN - 1)  # 104 output rows per tile
NT = (L + M - 1) // M  # 40 global tiles
TPC = NT // NCORES  # 5 tiles per core
N = B * C  # 2048 free dim
NW = M + N  # 2152: packed weight columns + x tile
CH = 512  # matmul moving free size (1 bank)
HALF = 1024  # drain granularity (2 banks)

F32 = mybir.dt.float32
F16 = mybir.dt.float16


def _host_weights(W: np.ndarray):
    o = np.arange(L)[:, None]
    p = np.arange(PADDED)[None, :]
    Wm = np.where((p >= o) & (p < o + WIN), W, 0.0).astype(np.float32)
    # wb[k, t, m] = Wm[t*104+m, t*104+k], zero-padded out of range
    wb = np.zeros((P, NT, M), np.float32)
    for t in range(NT):
        mt = min(M, L - t * M)
        kt = min(P, PADDED - t * M)
        wb[:kt, t, :mt] = Wm[t * M : t * M + mt, t * M : t * M + kt].T
    return wb.astype(np.float16)


def _host_x(x: np.ndarray):
    """x [B, L, C] f32 -> [P, NT, B, C] f16 in xpad-tile layout."""
    xp = np.concatenate([x[:, :PAD], x, x[:, -PAD:]], axis=1).astype(np.float16)
    xh = np.zeros((P, NT, B, C), np.float16)
    for t in range(NT):
        kt = min(P, PADDED - t * M)
        xh[:kt, t] = xp[:, t * M : t * M + kt].transpose(1, 0, 2)
    return xh


def _build_nc():
    nc = bacc.Bacc("TRN2", target_bir_lowering=False, debug=False, num_devices=NCORES)
    xwb_d = nc.dram_tensor("xwb", [P, TPC, NW], F16, kind="ExternalInput").ap()
    out_d = nc.dram_tensor("out", [M, TPC * N], F16, kind="ExternalOutput").ap()

    with tile.TileContext(nc) as tc:
        with (
            tc.tile_pool(name="main", bufs=1) as pool,
            tc.tile_pool(name="ps", bufs=4, space=bass.MemorySpace.PSUM) as pspool,
        ):
            xs = [pool.tile([P, NW], F16, name=f"xs{j}") for j in range(TPC)]
            # paired out tiles so DRAM write lines are 8KB (writes are slow)
            outs01 = pool.tile([M, 2 * N], F16, name="outs01")
            outs23 = pool.tile([M, 2 * N], F16, name="outs23")
            outs4 = pool.tile([M, N], F16, name="outs4")
            warm = pool.tile([P, CH], F16, name="warm")

            # tiles 0 and 4 split by partition across both HW rings
            nc.sync.dma_start(xs[0][: P // 2], xwb_d[: P // 2, 0])
            nc.scalar.dma_start(xs[0][P // 2 :], xwb_d[P // 2 :, 0])
            for j in (1, 2, 3):
                ring = nc.sync if j % 2 == 0 else nc.scalar
                ring.dma_start(xs[j][:], xwb_d[:, j])
            nc.sync.dma_start(xs[4][: P // 2], xwb_d[: P // 2, 4])
            nc.scalar.dma_start(xs[4][P // 2 :], xwb_d[P // 2 :, 4])

            # p-state warmup: keep the PE busy (rotating banks, no WAW gaps)
            # until the first x tile lands so real matmuls start at full clock
            nc.gpsimd.memset(warm[:], 0.0)
            for _ in range(5):
                wps = pspool.tile([M, HALF], F32, name="ps", tag="ps")
                nc.tensor.matmul(
                    wps[:, :CH], warm[:, :M], warm[:], start=True, stop=True
                )

            def out_view(j):
                if j < 2:
                    return outs01[:, j * N : (j + 1) * N]
                if j < 4:
                    return outs23[:, (j - 2) * N : (j - 1) * N]
                return outs4[:]

            di = 0
            for j in range(TPC):
                ov = out_view(j)
                for h in range(2):
                    ps = pspool.tile([M, HALF], F32, name="ps", tag="ps")
                    for c in range(2):
                        nc.tensor.matmul(
                            ps[:, c * CH : (c + 1) * CH],
                            xs[j][:, :M],
                            xs[j][:, M + h * HALF + c * CH : M + h * HALF + (c + 1) * CH],
                            start=True,
                            stop=True,
                        )
                    if di % 2 == 0:
                        nc.vector.tensor_scalar_add(
                            ov[:, h * HALF : (h + 1) * HALF], ps[:], 0.0
                        )
                    else:
                        nc.scalar.copy(ov[:, h * HALF : (h + 1) * HALF], ps[:])
                    di += 1
                if j == 1:
                    nc.gpsimd.dma_start(out_d[:, 0 : 2 * N], outs01[:])
                elif j == 3:
                    nc.sync.dma_start(out_d[:, 2 * N : 4 * N], outs23[:])
                elif j == 4:
                    # last tile: split BY PARTITION across both HW rings
                    nc.sync.dma_start(out_d[: M // 2, 4 * N :], outs4[: M // 2])
                    nc.scalar.dma_start(out_d[M // 2 :, 4 * N :], outs4[M // 2 :])

    nc.compile()
    return nc


_NC = None


def _get_nc():
    global _NC
    if _NC is None:
        _NC = _build_nc()
    return _NC


def _make_in_maps(x, W, b=None):
    wb = _host_weights(np.asarray(W, dtype=np.float32))  # [P, NT, M] f16
    xh = _host_x(np.asarray(x, dtype=np.float32))  # [P, NT, B, C] f16
    maps = []
    for s in range(NCORES):
        xwb = np.empty((P, TPC, NW), np.float16)
        xwb[:, :, :M] = wb[:, TPC * s : TPC * (s + 1)]
        xwb[:, :, M:] = xh[:, TPC * s : TPC * (s + 1)].reshape(P, TPC, N)
        maps.append({"xwb": xwb})
    return maps


def _gather(results, b):
    oh = np.concatenate(
        [r["out"].reshape(M, TPC, B, C) for r in results], axis=1
    )  # [104, 40, B, C]
    out = np.empty((B, L, C), np.float32)
    for t in range(NT):
        mt = min(M, L - t * M)
        out[:, t * M : t * M + mt] = oh[:mt, t].transpose(1, 0, 2)
    out += np.asarray(b, dtype=np.float32)[None, :, None]
    return out


def kernel(x: np.ndarray, W: np.ndarray, b: np.ndarray) -> np.ndarray:
    nc = _get_nc()
    res = run_bass_kernel_spmd(nc, _make_in_maps(x, W), list(range(NCORES)))
    return _gather(res.results, b)


if __name__ == "__main__":
    rng = np.random.default_rng(0)
    x = rng.standard_normal((B, L, C), dtype=np.float32)
    W = rng.standard_normal((L, PADDED), dtype=np.float32) * 0.02
    b = rng.standard_normal((L,), dtype=np.float32) * 0.02
    print(kernel(x, W, b).shape)


# revision 18
# speedup vs baseline: 1.0340x; 1.0340x over previous
"""Banded local-linear layer (nn_LocalLinearLayer) on 8 trn2 NeuronCores.

out[b, o, c] = sum_p W[o, p] * xpad[b, c, p] + bias[o],  band p in [o, o+25)
xpad = edge-replicate pad of x along L (first/last 12 rows duplicated).

Strategy (v11):
  - Tensor-parallel over L: 40 global output tiles of 104 rows (K=128 window);
    core s owns tiles [5s, 5s+5) and only its slice of the banded weight.
  - The per-tile weight block [128, 104] is PACKED at the head of the x tile
    ([104 w | 2048 x] = 4304 B lines), so each tile arrives in one large-line
    DMA. Tiles 0 and 4 are split BY PARTITION across both HW queues so the
    first matmul starts early and the input stream ends early.
  - Warmup matmuls on rotating PSUM banks keep the PE clock ramping before
    the first x tile lands.
  - Per tile: 4 matmuls (N=512, one PSUM bank each; bufs=8 via two pools of
    [104,1024]) drained per 1024 cols by a pure copy (vector/scalar
    alternating) into paired fp16 out tiles (8KB DRAM lines for the slow
    write path). Bias is added on the HOST during gather.
  - Out rings: tiles01 -> gpsimd, tiles23 -> sync, tile4 partition-split on
    sync+scalar.
  - fp16 operands and output, fp32 PSUM.
"""

import sys

for _p in ("/opt/trn_rl_repo",):
    if _p not in sys.path:
        sys.path.insert(0, _p)

import numpy as np

import concourse.bass as bass
import concourse.tile as tile
from concourse import bacc, mybir
from concourse.bass_utils import run_bass_kernel_spmd

L = 4096
WIN = 25
PAD = (WIN - 1) // 2  # 12
PADDED = L + 2 * PAD  # 4120
B = 32
C = 64
NCORES = 8
P = 128
M = P - (WIN - 1)  # 104 output rows per tile
NT = (L + M - 1) // M  # 40 global tiles
TPC = NT // NCORES  # 5 tiles per core
N = B * C  # 2048 free dim
NW = M + N  # 2152: packed weight columns + x tile
CH = 512  # matmul moving free size (1 bank)
HALF = 1024  # drain granularity (2 banks)

F32 = mybir.dt.float32
F16 = mybir.dt.float16


def _host_weights(W: np.ndarray):
    o = np.arange(L)[:, None]
    p = np.arange(PADDED)[None, :]
    Wm = np.where((p >= o) & (p < o + WIN), W, 0.0).astype(np.float32)
    # wb[k, t, m] = Wm[t*104+m, t*104+k], zero-padded out of range
    wb = np.zeros((P, NT, M), np.float32)
    for t in range(NT):
        mt = min(M, L - t * M)
        kt = min(P, PADDED - t * M)
        wb[:kt, t, :mt] = Wm[t * M : t * M + mt, t * M : t * M + kt].T
    return wb.astype(np.float16)


def _host_x(x: np.ndarray):
    """x [B, L, C] f32 -> [P, NT, B, C] f16 in xpad-tile layout."""
    xp = np.concatenate([x[:, :PAD], x, x[:, -PAD:]], axis=1).astype(np.float16)
    xh = np.zeros((P, NT, B, C), np.float16)
    for t in range(NT):
        kt = min(P, PADDED - t * M)
        xh[:kt, t] = xp[:, t * M : t * M + kt].transpose(1, 0, 2)
    return xh


def _build_nc():
    nc = bacc.Bacc("TRN2", target_bir_lowering=False, debug=False, num_devices=NCORES)
    xwb_d = nc.dram_tensor("xwb", [P, TPC, NW], F16, kind="ExternalInput").ap()
    out_d = nc.dram_tensor("out", [M, TPC * N], F16, kind="ExternalOutput").ap()

    with tile.TileContext(nc) as tc:
        with (
            tc.tile_pool(name="main", bufs=1) as pool,
            tc.tile_pool(name="ps", bufs=8, space=bass.MemorySpace.PSUM) as pspool,
        ):
            xs = [pool.tile([P, NW], F16, name=f"xs{j}") for j in range(TPC)]
            # paired out tiles so DRAM write lines are 8KB (writes are slow)
            outs01 = pool.tile([M, 2 * N], F16, name="outs01")
            outs23 = pool.tile([M, 2 * N], F16, name="outs23")
            outs4 = pool.tile([M, N], F16, name="outs4")
            warm = pool.tile([P, CH], F16, name="warm")

            # tiles 0 and 4 split by partition across both HW rings
            nc.sync.dma_start(xs[0][: P // 2], xwb_d[: P // 2, 0])
            nc.scalar.dma_start(xs[0][P // 2 :], xwb_d[P // 2 :, 0])
            for j in (1, 2, 3):
                ring = nc.sync if j % 2 == 0 else nc.scalar
                ring.dma_start(xs[j][:], xwb_d[:, j])
            nc.sync.dma_start(xs[4][: P // 2], xwb_d[: P // 2, 4])
            nc.scalar.dma_start(xs[4][P // 2 :], xwb_d[P // 2 :, 4])

            # p-state warmup: keep the PE busy (rotating banks, no WAW gaps)
            # until the first x tile lands so real matmuls start at full clock
            nc.gpsimd.memset(warm[:], 0.0)
            for _ in range(5):
                wps = pspool.tile([M, CH], F32, name="ps", tag="ps")
                nc.tensor.matmul(
                    wps[:], warm[:, :M], warm[:], start=True, stop=True
                )

            def out_view(j):
                if j < 2:
                    return outs01[:, j * N : (j + 1) * N]
                if j < 4:
                    return outs23[:, (j - 2) * N : (j - 1) * N]
                return outs4[:]

            di = 0
            for j in range(TPC):
                ov = out_view(j)
                for c in range(4):
                    ps = pspool.tile([M, CH], F32, name="ps", tag="ps")
                    nc.tensor.matmul(
                        ps[:],
                        xs[j][:, :M],
                        xs[j][:, M + c * CH : M + (c + 1) * CH],
                        start=True,
                        stop=True,
                    )
                    if di % 2 == 0:
                        nc.vector.tensor_scalar_add(
                            ov[:, c * CH : (c + 1) * CH], ps[:], 0.0
                        )
                    else:
                        nc.scalar.copy(ov[:, c * CH : (c + 1) * CH], ps[:])
                    di += 1
                if j == 1:
                    nc.gpsimd.dma_start(out_d[:, 0 : 2 * N], outs01[:])
                elif j == 3:
                    nc.sync.dma_start(out_d[:, 2 * N : 4 * N], outs23[:])
                elif j == 4:
                    # last tile: split BY PARTITION across both HW rings
                    nc.sync.dma_start(out_d[: M // 2, 4 * N :], outs4[: M // 2])
                    nc.scalar.dma_start(out_d[M // 2 :, 4 * N :], outs4[M // 2 :])

    nc.compile()
    return nc


_NC = None


def _get_nc():
    global _NC
    if _NC is None:
        _NC = _build_nc()
    return _NC


def _make_in_maps(x, W, b=None):
    wb = _host_weights(np.asarray(W, dtype=np.float32))  # [P, NT, M] f16
    xh = _host_x(np.asarray(x, dtype=np.float32))  # [P, NT, B, C] f16
    maps = []
    for s in range(NCORES):
        xwb = np.empty((P, TPC, NW), np.float16)
        xwb[:, :, :M] = wb[:, TPC * s : TPC * (s + 1)]
        xwb[:, :, M:] = xh[:, TPC * s : TPC * (s + 1)].reshape(P, TPC, N)
        maps.append({"xwb": xwb})
    return maps


def _gather(results, b):
    oh = np.concatenate(
        [r["out"].reshape(M, TPC, B, C) for r in results], axis=1
    )  # [104, 40, B, C]
    out = np.empty((B, L, C), np.float32)
    for t in range(NT):
        mt = min(M, L - t * M)
        out[:, t * M : t * M + mt] = oh[:mt, t].transpose(1, 0, 2)
    out += np.asarray(b, dtype=np.float32)[None, :, None]
    return out


def kernel(x: np.ndarray, W: np.ndarray, b: np.ndarray) -> np.ndarray:
    nc = _get_nc()
    res = run_bass_kernel_spmd(nc, _make_in_maps(x, W), list(range(NCORES)))
    return _gather(res.results, b)


if __name__ == "__main__":
    rng = np.random.default_rng(0)
    x = rng.standard_normal((B, L, C), dtype=np.float32)
    W = rng.standard_normal((L, PADDED), dtype=np.float32) * 0.02
    b = rng.standard_normal((L,), dtype=np.float32) * 0.02
    print(kernel(x, W, b).shape)


# revision 19
# speedup vs baseline: 1.0726x; 1.0373x over previous
"""Banded local-linear layer (nn_LocalLinearLayer) on 8 trn2 NeuronCores.

out[b, o, c] = sum_p W[o, p] * xpad[b, c, p] + bias[o],  band p in [o, o+25)
xpad = edge-replicate pad of x along L (first/last 12 rows duplicated).

Strategy (v13):
  - Tensor-parallel over L: 40 global output tiles of 104 rows (K=128 window);
    core s owns tiles [5s, 5s+5) and only its slice of the banded weight.
  - The per-tile weight block [128, 104] is PACKED at the head of the x tile
    ([104 w | 2048 x] = 4304 B lines): each tile arrives in one large-line DMA
    (sync ring: even tiles, scalar ring: odd tiles).
  - Gapless warmup matmuls on rotating PSUM banks bridge until the first x
    tile lands, so real matmuls run at full PE clock from the start.
  - Per tile: 4 matmuls (N=512, single-bank PSUM, bufs=8 so recycle latency
    never stalls), each drained by a pure copy (vector/scalar alternating)
    into fp16 out tiles. Bias is added on the HOST during gather.
  - Writes (slow: ~130 GB/s per queue at 4KB lines): out0/out2 on the gpsimd
    SW ring, out1 on sync; the late tiles 3 and 4 are partition-split three
    ways (sync/gpsimd/scalar) so the tail drains in ~1 us; scalar's triggers
    are emitted after all its drains.
  - fp16 operands and output, fp32 PSUM.
"""

import sys

for _p in ("/opt/trn_rl_repo",):
    if _p not in sys.path:
        sys.path.insert(0, _p)

import numpy as np

import concourse.bass as bass
import concourse.tile as tile
from concourse import bacc, mybir
from concourse.bass_utils import run_bass_kernel_spmd

L = 4096
WIN = 25
PAD = (WIN - 1) // 2  # 12
PADDED = L + 2 * PAD  # 4120
B = 32
C = 64
NCORES = 8
P = 128
M = P - (WIN - 1)  # 104 output rows per tile
NT = (L + M - 1) // M  # 40 global tiles
TPC = NT // NCORES  # 5 tiles per core
N = B * C  # 2048 free dim
NW = M + N  # 2152: packed weight columns + x tile
CH = 512  # matmul moving free size (1 bank)
NWARM = 7

F32 = mybir.dt.float32
F16 = mybir.dt.float16


def _host_weights(W: np.ndarray):
    o = np.arange(L)[:, None]
    p = np.arange(PADDED)[None, :]
    Wm = np.where((p >= o) & (p < o + WIN), W, 0.0).astype(np.float32)
    # wb[k, t, m] = Wm[t*104+m, t*104+k], zero-padded out of range
    wb = np.zeros((P, NT, M), np.float32)
    for t in range(NT):
        mt = min(M, L - t * M)
        kt = min(P, PADDED - t * M)
        wb[:kt, t, :mt] = Wm[t * M : t * M + mt, t * M : t * M + kt].T
    return wb.astype(np.float16)


def _host_x(x: np.ndarray):
    """x [B, L, C] f32 -> [P, NT, B, C] f16 in xpad-tile layout."""
    xp = np.concatenate([x[:, :PAD], x, x[:, -PAD:]], axis=1).astype(np.float16)
    xh = np.zeros((P, NT, B, C), np.float16)
    for t in range(NT):
        kt = min(P, PADDED - t * M)
        xh[:kt, t] = xp[:, t * M : t * M + kt].transpose(1, 0, 2)
    return xh


def _build_nc():
    nc = bacc.Bacc("TRN2", target_bir_lowering=False, debug=False, num_devices=NCORES)
    xwb_d = nc.dram_tensor("xwb", [P, TPC, NW], F16, kind="ExternalInput").ap()
    out_d = nc.dram_tensor("out", [M, TPC * N], F16, kind="ExternalOutput").ap()

    T3 = M // 3  # 34: partition split for the tail tiles

    with tile.TileContext(nc) as tc:
        with (
            tc.tile_pool(name="main", bufs=1) as pool,
            tc.tile_pool(name="ps", bufs=8, space=bass.MemorySpace.PSUM) as pspool,
        ):
            xs = [pool.tile([P, NW], F16, name=f"xs{j}") for j in range(TPC)]
            outs = [pool.tile([M, N], F16, name=f"outs{j}") for j in range(TPC)]
            warm = pool.tile([P, CH], F16, name="warm")

            for j in range(TPC):
                ring = nc.sync if j % 2 == 0 else nc.scalar
                ring.dma_start(xs[j][:], xwb_d[:, j])

            # p-state warmup: gapless matmuls on rotating PSUM banks until the
            # first x tile lands, so real matmuls start at full PE clock
            nc.gpsimd.memset(warm[:], 0.0)
            for _ in range(NWARM):
                wps = pspool.tile([M, CH], F32, name="ps", tag="ps")
                nc.tensor.matmul(
                    wps[:], warm[:, :M], warm[:], start=True, stop=True
                )

            di = 0
            for j in range(TPC):
                for c in range(4):
                    ps = pspool.tile([M, CH], F32, name="ps", tag="ps")
                    nc.tensor.matmul(
                        ps[:],
                        xs[j][:, :M],
                        xs[j][:, M + c * CH : M + (c + 1) * CH],
                        start=True,
                        stop=True,
                    )
                    if di % 2 == 0:
                        nc.vector.tensor_scalar_add(
                            outs[j][:, c * CH : (c + 1) * CH], ps[:], 0.0
                        )
                    else:
                        nc.scalar.copy(outs[j][:, c * CH : (c + 1) * CH], ps[:])
                    di += 1
                lo = j * N
                if j == 0 or j == 2:
                    nc.gpsimd.dma_start(out_d[:, lo : lo + N], outs[j][:])
                elif j == 1:
                    nc.sync.dma_start(out_d[:, lo : lo + N], outs[j][:])
                else:
                    # late tiles: partition-split so each queue's piece is ~1us
                    nc.sync.dma_start(out_d[:T3, lo : lo + N], outs[j][:T3])
                    nc.gpsimd.dma_start(
                        out_d[T3 : 2 * T3, lo : lo + N], outs[j][T3 : 2 * T3]
                    )
            # scalar's write triggers go after all its drains so descriptor
            # generation never delays the drain stream
            for j in (3, 4):
                lo = j * N
                nc.scalar.dma_start(out_d[2 * T3 :, lo : lo + N], outs[j][2 * T3 :])

    nc.compile()
    return nc


_NC = None


def _get_nc():
    global _NC
    if _NC is None:
        _NC = _build_nc()
    return _NC


def _make_in_maps(x, W, b=None):
    wb = _host_weights(np.asarray(W, dtype=np.float32))  # [P, NT, M] f16
    xh = _host_x(np.asarray(x, dtype=np.float32))  # [P, NT, B, C] f16
    maps = []
    for s in range(NCORES):
        xwb = np.empty((P, TPC, NW), np.float16)
        xwb[:, :, :M] = wb[:, TPC * s : TPC * (s + 1)]
        xwb[:, :, M:] = xh[:, TPC * s : TPC * (s + 1)].reshape(P, TPC, N)
        maps.append({"xwb": xwb})
    return maps


def _gather(results, b):
    oh = np.concatenate(
        [r["out"].reshape(M, TPC, B, C) for r in results], axis=1
    )  # [104, 40, B, C]
    out = np.empty((B, L, C), np.float32)
    for t in range(NT):
        mt = min(M, L - t * M)
        out[:, t * M : t * M + mt] = oh[:mt, t].transpose(1, 0, 2)
    out += np.asarray(b, dtype=np.float32)[None, :, None]
    return out


def kernel(x: np.ndarray, W: np.ndarray, b: np.ndarray) -> np.ndarray:
    nc = _get_nc()
    res = run_bass_kernel_spmd(nc, _make_in_maps(x, W), list(range(NCORES)))
    return _gather(res.results, b)


if __name__ == "__main__":
    rng = np.random.default_rng(0)
    x = rng.standard_normal((B, L, C), dtype=np.float32)
    W = rng.standard_normal((L, PADDED), dtype=np.float32) * 0.02
    b = rng.standard_normal((L,), dtype=np.float32) * 0.02
    print(kernel(x, W, b).shape)


# revision 20
# speedup vs baseline: 1.2230x; 1.1403x over previous
"""Banded local-linear layer (nn_LocalLinearLayer) on 8 trn2 NeuronCores.

out[b, o, c] = sum_p W[o, p] * xpad[b, c, p] + bias[o],  band p in [o, o+25)
xpad = edge-replicate pad of x along L (first/last 12 rows duplicated).

Strategy (v13):
  - Tensor-parallel over L: 40 global output tiles of 104 rows (K=128 window);
    core s owns tiles [5s, 5s+5) and only its slice of the banded weight.
  - The per-tile weight block [128, 104] is PACKED at the head of the x tile
    ([104 w | 2048 x] = 4304 B lines): each tile arrives in one large-line DMA
    (sync ring: even tiles, scalar ring: odd tiles).
  - Gapless warmup matmuls on rotating PSUM banks bridge until the first x
    tile lands, so real matmuls run at full PE clock from the start.
  - Per tile: 4 matmuls (N=512, single-bank PSUM, bufs=8 so recycle latency
    never stalls), each drained by a pure copy (vector/scalar alternating)
    into fp16 out tiles. Bias is added on the HOST during gather.
  - Writes (slow: ~130 GB/s per queue at 4KB lines): out0/out2 on the gpsimd
    SW ring, out1 on sync; the late tiles 3 and 4 are partition-split three
    ways (sync/gpsimd/scalar) so the tail drains in ~1 us; scalar's triggers
    are emitted after all its drains.
  - fp16 operands and output, fp32 PSUM.
"""

import sys

for _p in ("/opt/trn_rl_repo",):
    if _p not in sys.path:
        sys.path.insert(0, _p)

import numpy as np

import concourse.bass as bass
import concourse.tile as tile
from concourse import bacc, mybir
from concourse.bass_utils import run_bass_kernel_spmd

L = 4096
WIN = 25
PAD = (WIN - 1) // 2  # 12
PADDED = L + 2 * PAD  # 4120
B = 32
C = 64
NCORES = 8
P = 128
M = P - (WIN - 1)  # 104 output rows per tile
NT = (L + M - 1) // M  # 40 global tiles
TPC = NT // NCORES  # 5 tiles per core
N = B * C  # 2048 free dim
NW = M + N  # 2152: packed weight columns + x tile
CH = 512  # matmul moving free size (1 bank)
NWARM = 7

F32 = mybir.dt.float32
F16 = mybir.dt.float16


def _host_weights(W: np.ndarray):
    o = np.arange(L)[:, None]
    p = np.arange(PADDED)[None, :]
    Wm = np.where((p >= o) & (p < o + WIN), W, 0.0).astype(np.float32)
    # wb[k, t, m] = Wm[t*104+m, t*104+k], zero-padded out of range
    wb = np.zeros((P, NT, M), np.float32)
    for t in range(NT):
        mt = min(M, L - t * M)
        kt = min(P, PADDED - t * M)
        wb[:kt, t, :mt] = Wm[t * M : t * M + mt, t * M : t * M + kt].T
    return wb.astype(np.float16)


def _host_x(x: np.ndarray):
    """x [B, L, C] f32 -> [P, NT, B, C] f16 in xpad-tile layout."""
    xp = np.concatenate([x[:, :PAD], x, x[:, -PAD:]], axis=1).astype(np.float16)
    xh = np.zeros((P, NT, B, C), np.float16)
    for t in range(NT):
        kt = min(P, PADDED - t * M)
        xh[:kt, t] = xp[:, t * M : t * M + kt].transpose(1, 0, 2)
    return xh


def _build_nc():
    nc = bacc.Bacc("TRN2", target_bir_lowering=False, debug=False, num_devices=NCORES)
    xwb_d = nc.dram_tensor("xwb", [P, TPC, NW], F16, kind="ExternalInput").ap()
    out_d = nc.dram_tensor("out", [M, TPC * N], F16, kind="ExternalOutput").ap()

    with tile.TileContext(nc) as tc:
        with (
            tc.tile_pool(name="main", bufs=1) as pool,
            tc.tile_pool(name="ps", bufs=8, space=bass.MemorySpace.PSUM) as pspool,
        ):
            xs = [pool.tile([P, NW], F16, name=f"xs{j}") for j in range(TPC)]
            outs = [pool.tile([M, N], F16, name=f"outs{j}") for j in range(TPC)]
            warm = pool.tile([P, CH], F16, name="warm")

            for j in range(TPC):
                ring = nc.sync if j % 2 == 0 else nc.scalar
                ring.dma_start(xs[j][:], xwb_d[:, j])

            # p-state warmup: gapless matmuls on rotating PSUM banks until the
            # first x tile lands, so real matmuls start at full PE clock
            nc.gpsimd.memset(warm[:], 0.0)
            for _ in range(NWARM):
                wps = pspool.tile([M, CH], F32, name="ps", tag="ps")
                nc.tensor.matmul(
                    wps[:], warm[:, :M], warm[:], start=True, stop=True
                )

            di = 0
            for j in range(TPC):
                for c in range(4):
                    ps = pspool.tile([M, CH], F32, name="ps", tag="ps")
                    nc.tensor.matmul(
                        ps[:],
                        xs[j][:, :M],
                        xs[j][:, M + c * CH : M + (c + 1) * CH],
                        start=True,
                        stop=True,
                    )
                    if di % 2 == 0:
                        nc.vector.tensor_scalar_add(
                            outs[j][:, c * CH : (c + 1) * CH], ps[:], 0.0
                        )
                    else:
                        nc.scalar.copy(outs[j][:, c * CH : (c + 1) * CH], ps[:])
                    di += 1
                lo = j * N
                if j == 0 or j == 2:
                    nc.gpsimd.dma_start(out_d[:, lo : lo + N], outs[j][:])
                elif j == 1 or j == 3:
                    nc.sync.dma_start(out_d[:, lo : lo + N], outs[j][:])
                else:
                    # last tile: column-halves on both HW rings (full-partition
                    # DMAs keep all 13 engine stripes; 2KB lines are tolerable)
                    nc.sync.dma_start(out_d[:, lo : lo + N // 2], outs[j][:, : N // 2])
                    nc.scalar.dma_start(
                        out_d[:, lo + N // 2 : lo + N], outs[j][:, N // 2 :]
                    )

    nc.compile()
    return nc


_NC = None


def _get_nc():
    global _NC
    if _NC is None:
        _NC = _build_nc()
    return _NC


def _make_in_maps(x, W, b=None):
    wb = _host_weights(np.asarray(W, dtype=np.float32))  # [P, NT, M] f16
    xh = _host_x(np.asarray(x, dtype=np.float32))  # [P, NT, B, C] f16
    maps = []
    for s in range(NCORES):
        xwb = np.empty((P, TPC, NW), np.float16)
        xwb[:, :, :M] = wb[:, TPC * s : TPC * (s + 1)]
        xwb[:, :, M:] = xh[:, TPC * s : TPC * (s + 1)].reshape(P, TPC, N)
        maps.append({"xwb": xwb})
    return maps


def _gather(results, b):
    oh = np.concatenate(
        [r["out"].reshape(M, TPC, B, C) for r in results], axis=1
    )  # [104, 40, B, C]
    out = np.empty((B, L, C), np.float32)
    for t in range(NT):
        mt = min(M, L - t * M)
        out[:, t * M : t * M + mt] = oh[:mt, t].transpose(1, 0, 2)
    out += np.asarray(b, dtype=np.float32)[None, :, None]
    return out


def kernel(x: np.ndarray, W: np.ndarray, b: np.ndarray) -> np.ndarray:
    nc = _get_nc()
    res = run_bass_kernel_spmd(nc, _make_in_maps(x, W), list(range(NCORES)))
    return _gather(res.results, b)


if __name__ == "__main__":
    rng = np.random.default_rng(0)
    x = rng.standard_normal((B, L, C), dtype=np.float32)
    W = rng.standard_normal((L, PADDED), dtype=np.float32) * 0.02
    b = rng.standard_normal((L,), dtype=np.float32) * 0.02
    print(kernel(x, W, b).shape)


# revision 21
# speedup vs baseline: 1.2255x; 1.0020x over previous
"""Banded local-linear layer (nn_LocalLinearLayer) on 8 trn2 NeuronCores.

out[b, o, c] = sum_p W[o, p] * xpad[b, c, p] + bias[o],  band p in [o, o+25)
xpad = edge-replicate pad of x along L (first/last 12 rows duplicated).

Strategy (v13):
  - Tensor-parallel over L: 40 global output tiles of 104 rows (K=128 window);
    core s owns tiles [5s, 5s+5) and only its slice of the banded weight.
  - The per-tile weight block [128, 104] is PACKED at the head of the x tile
    ([104 w | 2048 x] = 4304 B lines): each tile arrives in one large-line DMA
    (sync ring: even tiles, scalar ring: odd tiles).
  - Gapless warmup matmuls on rotating PSUM banks bridge until the first x
    tile lands, so real matmuls run at full PE clock from the start.
  - Per tile: 4 matmuls (N=512, single-bank PSUM, bufs=8 so recycle latency
    never stalls), each drained by a pure copy (vector/scalar alternating)
    into fp16 out tiles. Bias is added on the HOST during gather.
  - Writes (slow: ~130 GB/s per queue at 4KB lines): out0/out2 on the gpsimd
    SW ring, out1 on sync; the late tiles 3 and 4 are partition-split three
    ways (sync/gpsimd/scalar) so the tail drains in ~1 us; scalar's triggers
    are emitted after all its drains.
  - fp16 operands and output, fp32 PSUM.
"""

import sys

for _p in ("/opt/trn_rl_repo",):
    if _p not in sys.path:
        sys.path.insert(0, _p)

import numpy as np
import ml_dtypes

import concourse.bass as bass
import concourse.tile as tile
from concourse import bacc, mybir
from concourse.bass_utils import run_bass_kernel_spmd

L = 4096
WIN = 25
PAD = (WIN - 1) // 2  # 12
PADDED = L + 2 * PAD  # 4120
B = 32
C = 64
NCORES = 8
P = 128
M = P - (WIN - 1)  # 104 output rows per tile
NT = (L + M - 1) // M  # 40 global tiles
TPC = NT // NCORES  # 5 tiles per core
N = B * C  # 2048 free dim
NW = M + N  # 2152: packed weight columns + x tile
CH = 512  # matmul moving free size (1 bank)
NWARM = 7

F32 = mybir.dt.float32
F16 = mybir.dt.float16
BF16 = mybir.dt.bfloat16


def _host_weights(W: np.ndarray):
    o = np.arange(L)[:, None]
    p = np.arange(PADDED)[None, :]
    Wm = np.where((p >= o) & (p < o + WIN), W, 0.0).astype(np.float32)
    # wb[k, t, m] = Wm[t*104+m, t*104+k], zero-padded out of range
    wb = np.zeros((P, NT, M), np.float32)
    for t in range(NT):
        mt = min(M, L - t * M)
        kt = min(P, PADDED - t * M)
        wb[:kt, t, :mt] = Wm[t * M : t * M + mt, t * M : t * M + kt].T
    return wb.astype(np.float16)


def _host_x(x: np.ndarray):
    """x [B, L, C] f32 -> [P, NT, B, C] f16 in xpad-tile layout."""
    xp = np.concatenate([x[:, :PAD], x, x[:, -PAD:]], axis=1).astype(np.float16)
    xh = np.zeros((P, NT, B, C), np.float16)
    for t in range(NT):
        kt = min(P, PADDED - t * M)
        xh[:kt, t] = xp[:, t * M : t * M + kt].transpose(1, 0, 2)
    return xh


def _build_nc():
    nc = bacc.Bacc("TRN2", target_bir_lowering=False, debug=False, num_devices=NCORES)
    xwb_d = nc.dram_tensor("xwb", [P, TPC, NW], BF16, kind="ExternalInput").ap()
    out_d = nc.dram_tensor("out", [M, TPC * N], F16, kind="ExternalOutput").ap()

    with tile.TileContext(nc) as tc:
        with (
            tc.tile_pool(name="main", bufs=1) as pool,
            tc.tile_pool(name="ps", bufs=8, space=bass.MemorySpace.PSUM) as pspool,
        ):
            xs = [pool.tile([P, NW], BF16, name=f"xs{j}") for j in range(TPC)]
            outs = [pool.tile([M, N], F16, name=f"outs{j}") for j in range(TPC)]
            warm = pool.tile([P, CH], BF16, name="warm")

            for j in range(TPC):
                ring = nc.sync if j % 2 == 0 else nc.scalar
                ring.dma_start(xs[j][:], xwb_d[:, j])

            # p-state warmup: gapless matmuls on rotating PSUM banks until the
            # first x tile lands, so real matmuls start at full PE clock
            nc.gpsimd.memset(warm[:], 0.0)
            for _ in range(NWARM):
                wps = pspool.tile([M, CH], F32, name="ps", tag="ps")
                nc.tensor.matmul(
                    wps[:], warm[:, :M], warm[:], start=True, stop=True
                )

            di = 0
            for j in range(TPC):
                for c in range(4):
                    ps = pspool.tile([M, CH], F32, name="ps", tag="ps")
                    nc.tensor.matmul(
                        ps[:],
                        xs[j][:, :M],
                        xs[j][:, M + c * CH : M + (c + 1) * CH],
                        start=True,
                        stop=True,
                    )
                    if di % 2 == 0:
                        nc.vector.tensor_scalar_add(
                            outs[j][:, c * CH : (c + 1) * CH], ps[:], 0.0
                        )
                    else:
                        nc.scalar.copy(outs[j][:, c * CH : (c + 1) * CH], ps[:])
                    di += 1
                lo = j * N
                if j == 0 or j == 2:
                    nc.gpsimd.dma_start(out_d[:, lo : lo + N], outs[j][:])
                elif j == 1 or j == 3:
                    nc.sync.dma_start(out_d[:, lo : lo + N], outs[j][:])
                else:
                    # last tile: column-halves on both HW rings (full-partition
                    # DMAs keep all 13 engine stripes; 2KB lines are tolerable)
                    nc.sync.dma_start(out_d[:, lo : lo + N // 2], outs[j][:, : N // 2])
                    nc.scalar.dma_start(
                        out_d[:, lo + N // 2 : lo + N], outs[j][:, N // 2 :]
                    )

    nc.compile()
    return nc


_NC = None


def _get_nc():
    global _NC
    if _NC is None:
        _NC = _build_nc()
    return _NC


def _make_in_maps(x, W, b=None):
    wb = _host_weights(np.asarray(W, dtype=np.float32))  # [P, NT, M] f16
    xh = _host_x(np.asarray(x, dtype=np.float32))  # [P, NT, B, C] f16
    maps = []
    for s in range(NCORES):
        xwb = np.empty((P, TPC, NW), ml_dtypes.bfloat16)
        xwb[:, :, :M] = wb[:, TPC * s : TPC * (s + 1)]
        xwb[:, :, M:] = xh[:, TPC * s : TPC * (s + 1)].reshape(P, TPC, N)
        maps.append({"xwb": xwb})
    return maps


def _gather(results, b):
    oh = np.concatenate(
        [r["out"].reshape(M, TPC, B, C) for r in results], axis=1
    )  # [104, 40, B, C]
    out = np.empty((B, L, C), np.float32)
    for t in range(NT):
        mt = min(M, L - t * M)
        out[:, t * M : t * M + mt] = oh[:mt, t].transpose(1, 0, 2)
    out += np.asarray(b, dtype=np.float32)[None, :, None]
    return out


def kernel(x: np.ndarray, W: np.ndarray, b: np.ndarray) -> np.ndarray:
    nc = _get_nc()
    res = run_bass_kernel_spmd(nc, _make_in_maps(x, W), list(range(NCORES)))
    return _gather(res.results, b)


if __name__ == "__main__":
    rng = np.random.default_rng(0)
    x = rng.standard_normal((B, L, C), dtype=np.float32)
    W = rng.standard_normal((L, PADDED), dtype=np.float32) * 0.02
    b = rng.standard_normal((L,), dtype=np.float32) * 0.02
    print(kernel(x, W, b).shape)


# revision 22
# speedup vs baseline: 1.2509x; 1.0207x over previous
"""Banded local-linear layer (nn_LocalLinearLayer) on 8 trn2 NeuronCores.

out[b, o, c] = sum_p W[o, p] * xpad[b, c, p] + bias[o],  band p in [o, o+25)
xpad = edge-replicate pad of x along L (first/last 12 rows duplicated).

Strategy (v14):
  - Tensor-parallel over L: 40 global output tiles of 104 rows (K=128 window);
    core s owns tiles [5s, 5s+5) and only its slice of the banded weight.
  - The per-tile weight block [128, 104] is PACKED at the head of the x tile
    ([104 w | 2048 x] = 4304 B lines): each tile arrives in one large-line DMA
    (sync ring: even tiles, scalar ring: odd tiles).
  - Gapless warmup matmuls on rotating PSUM banks bridge until the first x
    tile lands, so real matmuls run at full PE clock from the start.
  - Per tile: 4 matmuls (N=512, single-bank PSUM, bufs=8 so recycle latency
    never stalls), each drained by a pure copy (vector/scalar alternating)
    into fp16 out tiles. Bias is added on the HOST during gather.
  - Writes (slow, ~130 GB/s per queue at 4KB lines; full-partition DMAs only
    -- fewer partitions means fewer engine stripes): out0/out2 on the gpsimd
    SW ring, out1/out3 on sync, and the last tile in column-halves on both HW
    rings to shorten the tail.
  - fp16 operands and output, fp32 PSUM.
"""

import sys

for _p in ("/opt/trn_rl_repo",):
    if _p not in sys.path:
        sys.path.insert(0, _p)

import numpy as np

import concourse.bass as bass
import concourse.tile as tile
from concourse import bacc, mybir
from concourse.bass_utils import run_bass_kernel_spmd

L = 4096
WIN = 25
PAD = (WIN - 1) // 2  # 12
PADDED = L + 2 * PAD  # 4120
B = 32
C = 64
NCORES = 8
P = 128
M = P - (WIN - 1)  # 104 output rows per tile
NT = (L + M - 1) // M  # 40 global tiles
TPC = NT // NCORES  # 5 tiles per core
N = B * C  # 2048 free dim
NW = M + N  # 2152: packed weight columns + x tile
CH = 512  # matmul moving free size (1 bank)
NWARM = 7

F32 = mybir.dt.float32
F16 = mybir.dt.float16


def _host_weights(W: np.ndarray):
    o = np.arange(L)[:, None]
    p = np.arange(PADDED)[None, :]
    Wm = np.where((p >= o) & (p < o + WIN), W, 0.0).astype(np.float32)
    # wb[k, t, m] = Wm[t*104+m, t*104+k], zero-padded out of range
    wb = np.zeros((P, NT, M), np.float32)
    for t in range(NT):
        mt = min(M, L - t * M)
        kt = min(P, PADDED - t * M)
        wb[:kt, t, :mt] = Wm[t * M : t * M + mt, t * M : t * M + kt].T
    return wb.astype(np.float16)


def _host_x(x: np.ndarray):
    """x [B, L, C] f32 -> [P, NT, B, C] f16 in xpad-tile layout."""
    xp = np.concatenate([x[:, :PAD], x, x[:, -PAD:]], axis=1).astype(np.float16)
    xh = np.zeros((P, NT, B, C), np.float16)
    for t in range(NT):
        kt = min(P, PADDED - t * M)
        xh[:kt, t] = xp[:, t * M : t * M + kt].transpose(1, 0, 2)
    return xh


def _build_nc():
    nc = bacc.Bacc("TRN2", target_bir_lowering=False, debug=False, num_devices=NCORES)
    xwb_d = nc.dram_tensor("xwb", [P, TPC, NW], F16, kind="ExternalInput").ap()
    out_d = nc.dram_tensor("out", [M, TPC * N], F16, kind="ExternalOutput").ap()

    with tile.TileContext(nc) as tc:
        with (
            tc.tile_pool(name="main", bufs=1) as pool,
            tc.tile_pool(name="ps", bufs=8, space=bass.MemorySpace.PSUM) as pspool,
        ):
            xs = [pool.tile([P, NW], F16, name=f"xs{j}") for j in range(TPC)]
            outs = [pool.tile([M, N], F16, name=f"outs{j}") for j in range(TPC)]
            warm = pool.tile([P, CH], F16, name="warm")

            for j in range(TPC):
                ring = nc.sync if j % 2 == 0 else nc.scalar
                ring.dma_start(xs[j][:], xwb_d[:, j])

            # p-state warmup: gapless matmuls on rotating PSUM banks until the
            # first x tile lands, so real matmuls start at full PE clock
            nc.gpsimd.memset(warm[:], 0.0)
            for _ in range(NWARM):
                wps = pspool.tile([M, CH], F32, name="ps", tag="ps")
                nc.tensor.matmul(
                    wps[:], warm[:, :M], warm[:], start=True, stop=True
                )

            di = 0
            for j in range(TPC):
                for c in range(4):
                    ps = pspool.tile([M, CH], F32, name="ps", tag="ps")
                    nc.tensor.matmul(
                        ps[:],
                        xs[j][:, :M],
                        xs[j][:, M + c * CH : M + (c + 1) * CH],
                        start=True,
                        stop=True,
                    )
                    if di % 2 == 0:
                        nc.vector.tensor_scalar_add(
                            outs[j][:, c * CH : (c + 1) * CH], ps[:], 0.0
                        )
                    else:
                        nc.scalar.copy(outs[j][:, c * CH : (c + 1) * CH], ps[:])
                    di += 1
                lo = j * N
                if j == 0 or j == 2:
                    nc.gpsimd.dma_start(out_d[:, lo : lo + N], outs[j][:])
                elif j == 1 or j == 3:
                    nc.sync.dma_start(out_d[:, lo : lo + N], outs[j][:])
                else:
                    # last tile: column-halves on both HW rings (full-partition
                    # DMAs keep all 13 engine stripes; 2KB lines are tolerable)
                    nc.sync.dma_start(out_d[:, lo : lo + N // 2], outs[j][:, : N // 2])
                    nc.scalar.dma_start(
                        out_d[:, lo + N // 2 : lo + N], outs[j][:, N // 2 :]
                    )

    nc.compile()
    return nc


_NC = None


def _get_nc():
    global _NC
    if _NC is None:
        _NC = _build_nc()
    return _NC


def _make_in_maps(x, W, b=None):
    wb = _host_weights(np.asarray(W, dtype=np.float32))  # [P, NT, M] f16
    xh = _host_x(np.asarray(x, dtype=np.float32))  # [P, NT, B, C] f16
    maps = []
    for s in range(NCORES):
        xwb = np.empty((P, TPC, NW), np.float16)
        xwb[:, :, :M] = wb[:, TPC * s : TPC * (s + 1)]
        xwb[:, :, M:] = xh[:, TPC * s : TPC * (s + 1)].reshape(P, TPC, N)
        maps.append({"xwb": xwb})
    return maps


def _gather(results, b):
    oh = np.concatenate(
        [r["out"].reshape(M, TPC, B, C) for r in results], axis=1
    )  # [104, 40, B, C]
    out = np.empty((B, L, C), np.float32)
    for t in range(NT):
        mt = min(M, L - t * M)
        out[:, t * M : t * M + mt] = oh[:mt, t].transpose(1, 0, 2)
    out += np.asarray(b, dtype=np.float32)[None, :, None]
    return out


def kernel(x: np.ndarray, W: np.ndarray, b: np.ndarray) -> np.ndarray:
    nc = _get_nc()
    res = run_bass_kernel_spmd(nc, _make_in_maps(x, W), list(range(NCORES)))
    return _gather(res.results, b)


if __name__ == "__main__":
    rng = np.random.default_rng(0)
    x = rng.standard_normal((B, L, C), dtype=np.float32)
    W = rng.standard_normal((L, PADDED), dtype=np.float32) * 0.02
    b = rng.standard_normal((L,), dtype=np.float32) * 0.02
    print(kernel(x, W, b).shape)
